# revision 19
# baseline (speedup 1.0000x reference)
"""Trainium2 Bass kernel for nn_BasicBlock (binary activation + binarized
weight-standardized 3x3 conv + residual + PReLU).

Contract: kernel(**inputs) takes FULL unsharded numpy inputs (keys as in
setup_inputs) and returns the FULL [32, 512, 28, 28] float32 output.
Internally shards the batch dim across 8 NeuronCores (4 images each); the
small conv weight + per-channel vectors are replicated.

Key math facts exploited:
- forward activations are sign(x*beta+b0) in {-1,0,1} and forward weights
  are sf[o]*gain[o]*sign(w_std) with sign in {-1,0,1}; we encode acts as
  +-0.5 (is_gt trick on DVE/gpsimd, off the ACT engine) and weights as
  +-1, folding the 2x into the per-channel epilogue scalar alphabar.
- fp8e4 DoubleRow packs two contraction rows per PE cell (2 cin chunks per
  matmul), halving the matmul count; products are +-0.5 so fp32 PSUM
  accumulation is exact.
- startup: x(image 0)+w(chunk 0) DMAs are prioritized, junk transposes
  warm the PE HAM clock gate, and the weight-prep pipeline
  (stats -> sign -> transpose -> cast) is pipelined at (tap, pair)
  granularity so conv starts as soon as the first weight tap is ready.
"""

import numpy as np

import concourse.bass as bass
import concourse.mybir as mybir
import concourse.tile as tile
from concourse import bacc
from concourse.masks import make_identity

# problem constants (hardcoded per harness contract)
N_CORES = 8
N_PER = 4          # images per core (32 / 8)
C = 512            # Cin == Cout
H = W = 28
HP = WP = 30       # zero-padded spatial
TAPS = 9
KFAN = C * TAPS    # 4608 = fan-in per output channel
ALPHA = 0.2
BETA = 1.0
EPS = 1e-5
WS_SCALE = 1.0 / float(np.sqrt(KFAN))  # fan_in**-0.5
NCH = C // 128     # 4 channel chunks of 128
NPAIR = NCH // 2   # 2 DoubleRow pairs of chunks
ROWS_PER_TILE = 14 # output rows per matmul tile
NSPAT = H // ROWS_PER_TILE  # 2 spatial tiles per image
NFREE = ROWS_PER_TILE * WP  # 420: contiguous run incl. 2 pad cols per row
ACT_IMG = 912  # padded 30x30 image (900) + 12 slack: %16==0 for DoubleRow,
               # and covers the last tile's 420-run overhang (482+420=902)
NROUND = NPAIR * TAPS  # 18 accumulation rounds per output tile

FP32 = mybir.dt.float32
BF16 = mybir.dt.bfloat16
FP8 = mybir.dt.float8e4


def build_program():
    nc = bacc.Bacc(
        "TRN2",
        target_bir_lowering=False,
        debug=False,
        num_devices=1,
        num_swdge_queues=4,
    )
    x_h = nc.declare_dram_parameter("x", [N_PER, C, H, W], FP32, isOutput=False)
    w_h = nc.declare_dram_parameter("conv_weight", [C, C, 3, 3], FP32, isOutput=False)
    gain_h = nc.declare_dram_parameter("gain", [C], FP32, isOutput=False)
    b0_h = nc.declare_dram_parameter("move0_bias", [C], FP32, isOutput=False)
    b1_h = nc.declare_dram_parameter("move1_bias", [C], FP32, isOutput=False)
    pa_h = nc.declare_dram_parameter("prelu_a", [C], FP32, isOutput=False)
    b2_h = nc.declare_dram_parameter("move2_bias", [C], FP32, isOutput=False)
    out_h = nc.declare_dram_parameter("out", [N_PER, C, H, W], FP32, isOutput=True)

    x_ap = x_h[:, :, :, :]
    w_ap = w_h[:, :, :, :]
    out_ap = out_h[:, :, :, :]

    with tile.TileContext(nc) as tc:
        with (
            tc.tile_pool(name="persist", bufs=1) as persist,
            tc.tile_pool(name="scratch", bufs=2) as scratch,
            tc.tile_pool(name="stats", bufs=4) as stats,
            tc.tile_pool(name="epi", bufs=4) as epi,
            tc.tile_pool(name="psum_mm", bufs=6, space="PSUM") as psum_mm,
            tc.tile_pool(name="psum_tr", bufs=2, space="PSUM") as psum_tr,
        ):
            # ---- identity for PE transposes + HAM warm-up ---------------
            ident = persist.tile([128, 128], BF16, tag="ident")
            make_identity(nc, ident)

            # ---- activation image tiles + memsets (border zeros) --------
            # act_img[q][n] : [128, 2, ACT_IMG] fp8 -- two cin chunks per
            # DoubleRow pair; zeros at spatial border.
            # ---- small per-channel vectors first: cheap swdge DMAs -------
            def load_vec(h, nm):
                t = persist.tile([128, NCH], FP32, tag=f"v_{nm}", name=f"v_{nm}")
                nc.gpsimd.dma_start(
                    out=t, in_=h[:].rearrange("(c p) -> p c", p=128)
                )
                return t

            gain_v = load_vec(gain_h, "gain")
            b0_v = load_vec(b0_h, "b0")
            b1_v = load_vec(b1_h, "b1")
            pa_v = load_vec(pa_h, "pa")
            b2_v = load_vec(b2_h, "b2")
            gain_c = [gain_v[:, c : c + 1] for c in range(NCH)]
            b1_c = [b1_v[:, c : c + 1] for c in range(NCH)]
            pa_c = [pa_v[:, c : c + 1] for c in range(NCH)]
            b2_c = [b2_v[:, c : c + 1] for c in range(NCH)]

            # thr[c] = -b0[c]/beta : act sign threshold (x > thr)
            thr_v = persist.tile([128, NCH], FP32, tag="thr")
            nc.vector.tensor_scalar_mul(out=thr_v, in0=b0_v, scalar1=-1.0 / BETA)
            thr_c = [thr_v[:, c : c + 1] for c in range(NCH)]

            # derived per-channel epilogue constants:
            #   one_minus_a = 1 - prelu_a ; ab1b2 = prelu_a*move1_bias + move2_bias
            oma_v = persist.tile([128, NCH], FP32, tag="oma")
            nc.vector.tensor_scalar(
                out=oma_v, in0=pa_v, scalar1=-1.0, scalar2=1.0,
                op0=mybir.AluOpType.mult, op1=mybir.AluOpType.add,
            )
            one_minus_a = [oma_v[:, c : c + 1] for c in range(NCH)]
            ab_v = persist.tile([128, NCH], FP32, tag="ab1b2")
            for c in range(NCH):
                nc.vector.scalar_tensor_tensor(
                    out=ab_v[:, c : c + 1], in0=b1_c[c], scalar=pa_c[c],
                    in1=b2_c[c],
                    op0=mybir.AluOpType.mult, op1=mybir.AluOpType.add,
                )
            ab1b2 = [ab_v[:, c : c + 1] for c in range(NCH)]

            # ---- activation image tiles + memsets (border zeros) ---------
            # act_img[q][n] : [128, 2, ACT_IMG] fp8 -- two cin chunks per
            # DoubleRow pair; zeros at spatial border. Image 0/1 tiles
            # memset first (they gate the first xsigns).
            act_img = [[None] * N_PER for _ in range(NPAIR)]
            for n in range(N_PER):
                for q in range(NPAIR):
                    act_img[q][n] = persist.tile(
                        [128, 2, ACT_IMG], FP8, tag=f"act{q}_{n}", name=f"act{q}_{n}"
                    )
            for n in range(N_PER):
                for q in range(NPAIR):
                    nc.gpsimd.memset(act_img[q][n], 0.0)

            # ---- HAM warm-up: junk transposes keep PE busy pre-conv ------
            # (shares the psum_tr rotation via the same tag)
            junk_ps = psum_tr.tile([128, 2, 128], BF16, tag="ptr", name="junk")
            for _ in range(100):
                nc.tensor.transpose(junk_ps[:, 0, :], ident, ident)

            # ---- weight DMAs: chunk 0 split into 9 sg-aligned pieces
            # across the two HWDGE engines; chunks 1-3 as single DMAs -----
            w_flat = w_ap.rearrange("o i a b -> o (i a b)")
            w_tiles = [None] * NCH

            xs_tiles = [
                persist.tile([128, N_PER, H, W], FP32, tag=f"xs{c}", name=f"xs{c}")
                for c in range(NCH)
            ]
            xr = x_ap.rearrange("n c h w -> c n h w")

            def x_dma(eng, c, n0, n1):
                eng.dma_start(
                    out=xs_tiles[c][:, n0:n1],
                    in_=xr[c * 128 : (c + 1) * 128, n0:n1],
                )

            def w_dma_chunk(m):
                w_tiles[m] = scratch.tile(
                    [128, KFAN], FP32, tag="wtile", name=f"wt{m}"
                )
                nc.sync.dma_start(
                    out=w_tiles[m],
                    in_=w_flat[m * 128 : (m + 1) * 128, :],
                )

            # hw-queue priority order (sync engine = issue order):
            # x(img0 pair0), w0 pieces, x(img1 pair0), x(img0/1 pair1),
            # w1, x(img2/3), w2, w3.  w0's pieces 5-8 go via scalar so the
            # two HWDGE engines issue concurrently.
            w_tiles[0] = scratch.tile([128, KFAN], FP32, tag="wtile", name="wt0")
            x_dma(nc.sync, 0, 0, 1)
            x_dma(nc.sync, 1, 0, 1)
            for j in range(TAPS):
                eng = nc.sync if j < 5 else nc.scalar
                eng.dma_start(
                    out=w_tiles[0][:, j * 512 : (j + 1) * 512],
                    in_=w_flat[0:128, j * 512 : (j + 1) * 512],
                )
            x_dma(nc.sync, 0, 1, 2)
            x_dma(nc.sync, 1, 1, 2)
            x_dma(nc.sync, 2, 0, 2)
            x_dma(nc.sync, 3, 0, 2)
            w_dma_chunk(1)
            x_dma(nc.sync, 0, 2, 4)
            x_dma(nc.sync, 1, 2, 4)
            x_dma(nc.sync, 2, 2, 4)
            x_dma(nc.sync, 3, 2, 4)
            w_dma_chunk(2)
            w_dma_chunk(3)

            # ---- act sign: sign(beta*x + b0) -> fp8 on ACT ---------------
            b0_c = [b0_v[:, c : c + 1] for c in range(NCH)]

            def xsign(n, c):
                dst = act_img[c // 2][n][:, c % 2, : HP * WP].rearrange(
                    "p (h w) -> p h w", w=WP
                )[:, 1 : 1 + H, 1 : 1 + W]
                nc.scalar.activation(
                    out=dst,
                    in_=xs_tiles[c][:, n],
                    func=mybir.ActivationFunctionType.Sign,
                    bias=b0_c[c],
                    scale=BETA,
                )

            # lhsT : [128(cin), tap, pair, half, cout] fp8 DoubleRow weights
            lhsT = persist.tile(
                [128, TAPS, NPAIR, 2, C], FP8, tag="lhsT", name="lhsT"
            )
            alphabar = {}  # per cout chunk [128,1]: 2*alpha*sf*gain
            wsigns = {}
            mvs = {}

            def weight_prep_a(m):
                """stats -> negmean: the critical path to the signs"""
                wt = w_tiles[m]
                st = stats.tile([128, TAPS, 6], FP32, tag="bnst", name="bnst")
                wt3 = wt.rearrange("p (a b) -> p a b", b=512)
                for sg in range(TAPS):
                    nc.vector.bn_stats(out=st[:, sg, :], in_=wt3[:, sg, :])
                mv = stats.tile([128, 2], FP32, tag="bnagg", name="bnagg")
                nc.vector.bn_aggr(out=mv, in_=st)

                negmean = stats.tile([128, 1], FP32, tag="negmean", name="negmean")
                nc.vector.tensor_scalar_mul(out=negmean, in0=mv[:, 0:1], scalar1=-1.0)
                mvs[m] = (mv, negmean)

            def weight_sign(m, q):
                """sign(w - mean) -> bf16 (+-1) for pair q's two cin blocks"""
                wt = w_tiles[m]
                _, negmean = mvs[m]
                if q == 0:
                    ws = scratch.tile([128, KFAN], BF16, tag="wsign", name="wsign")
                    wsigns[m] = ws
                ws = wsigns[m]
                for h2 in range(2):
                    b = 2 * q + h2
                    nc.scalar.activation(
                        out=ws[:, b * 1152 : (b + 1) * 1152],
                        in_=wt[:, b * 1152 : (b + 1) * 1152],
                        func=mybir.ActivationFunctionType.Sign,
                        bias=negmean,
                    )

            def weight_transpose(m, q):
                """per tap: 2 block transposes -> 1 fp8 cast (pair q)"""
                ws3 = wsigns[m].rearrange("p (i t) -> p i t", t=TAPS)
                for t in range(TAPS):
                    ps = psum_tr.tile(
                        [128, 2, 128], BF16, tag="ptr", name="ptr"
                    )
                    for h2 in range(2):
                        b = 2 * q + h2
                        nc.tensor.transpose(
                            ps[:, h2, :],
                            ws3[:, b * 128 : (b + 1) * 128, t],
                            ident,
                        )
                    nc.vector.tensor_copy(
                        out=lhsT[:, t, q, :, m * 128 : (m + 1) * 128],
                        in_=ps,
                    )

            def weight_prep_c(m):
                """1/(std+eps), sum|w-mean| -> alphabar; off critical path"""
                wt = w_tiles[m]
                mv, negmean = mvs[m]
                stdeps = stats.tile([128, 1], FP32, tag="stdeps", name="stdeps")
                nc.scalar.activation(
                    out=stdeps, in_=mv[:, 1:2], func=mybir.ActivationFunctionType.Sqrt
                )
                nc.vector.tensor_scalar_add(out=stdeps, in0=stdeps, scalar1=EPS)
                inv = stats.tile([128, 1], FP32, tag="inv", name="inv")
                nc.vector.reciprocal(out=inv, in_=stdeps)

                sumabs = stats.tile([128, NCH], FP32, tag="sumabs", name="sumabs")
                for b in range(NCH):
                    nc.scalar.activation(
                        out=wt[:, b * 1152 : (b + 1) * 1152],
                        in_=wt[:, b * 1152 : (b + 1) * 1152],
                        func=mybir.ActivationFunctionType.Abs,
                        bias=negmean,
                        accum_out=sumabs[:, b : b + 1],
                    )
                sumabs1 = stats.tile([128, 1], FP32, tag="sumabs1", name="sumabs1")
                nc.vector.tensor_reduce(
                    out=sumabs1, in_=sumabs, axis=mybir.AxisListType.X,
                    op=mybir.AluOpType.add,
                )

                ab = persist.tile(
                    [128, 1], FP32, tag=f"alphabar{m}", name=f"alphabar{m}"
                )
                nc.vector.tensor_tensor(
                    out=ab, in0=sumabs1, in1=inv, op=mybir.AluOpType.mult
                )
                nc.vector.tensor_tensor(
                    out=ab, in0=ab, in1=gain_c[m], op=mybir.AluOpType.mult
                )
                nc.vector.tensor_scalar_mul(
                    out=ab, in0=ab, scalar1=ALPHA * WS_SCALE / KFAN
                )
                alphabar[m] = ab

            def epilogue(m, n, h2, acc):
                y0 = h2 * ROWS_PER_TILE
                accv = acc.rearrange("p (h w) -> p h w", w=WP)[:, :, 0:W]
                res = xs_tiles[m][:, n, y0 : y0 + ROWS_PER_TILE, :]
                # z = acc*alphabar + residual   (prelu input minus b1)
                z = epi.tile(
                    [128, ROWS_PER_TILE, W], FP32, tag="z", name="z"
                )
                nc.vector.scalar_tensor_tensor(
                    out=z, in0=accv, scalar=alphabar[m], in1=res,
                    op0=mybir.AluOpType.mult, op1=mybir.AluOpType.add,
                )
                # r = relu(z + b1) on ACT
                r = epi.tile(
                    [128, ROWS_PER_TILE, W], FP32, tag="r", name="r"
                )
                nc.scalar.activation(
                    out=r, in_=z,
                    func=mybir.ActivationFunctionType.Relu,
                    bias=b1_c[m],
                )
                # zz = a*z + (a*b1 + b2) ; out = (1-a)*r + zz
                zz = epi.tile(
                    [128, ROWS_PER_TILE, W], FP32, tag="zz", name="zz"
                )
                nc.scalar.activation(
                    out=zz, in_=z,
                    func=mybir.ActivationFunctionType.Identity,
                    scale=pa_c[m], bias=ab1b2[m],
                )
                nc.vector.scalar_tensor_tensor(
                    out=zz, in0=r, scalar=one_minus_a[m], in1=zz,
                    op0=mybir.AluOpType.mult, op1=mybir.AluOpType.add,
                )
                nc.sync.dma_start(
                    out=out_ap[
                        n, m * 128 : (m + 1) * 128,
                        y0 : y0 + ROWS_PER_TILE, :,
                    ],
                    in_=zz,
                )

            def conv_rounds(m, group, accs, q):
                """emit the 9 accumulation rounds of pair q for a group"""
                for t in range(TAPS):
                    dy, dx = t // 3, t % 3
                    wslice = lhsT[:, t, q, :, m * 128 : (m + 1) * 128]
                    for j, (n, h2) in enumerate(group):
                        base = (h2 * ROWS_PER_TILE + dy) * WP + dx
                        rhs = act_img[q][n][:, :, base : base + NFREE]
                        nc.tensor.matmul(
                            accs[j],
                            wslice,
                            rhs,
                            start=(q == 0 and t == 0),
                            stop=(q == NPAIR - 1 and t == TAPS - 1),
                            perf_mode=mybir.MatmulPerfMode.DoubleRow,
                        )

            def conv_group(m, group):
                """one PSUM-resident group: 18 rounds, weight reused
                across the group's tiles within each round"""
                accs = [
                    psum_mm.tile([128, NFREE], FP32, tag="acc", name="acc")
                    for _ in group
                ]
                for q in range(NPAIR):
                    conv_rounds(m, group, accs, q)
                for j, (n, h2) in enumerate(group):
                    epilogue(m, n, h2, accs[j])

            # ---- chunk 0 startup pipeline: conv on image 0 starts as
            # soon as pair-0 weights + image-0 pair-0 acts are ready; the
            # pair-1 weight prep and remaining xsigns hide under the q0
            # conv rounds. ------------------------------------------------
            weight_prep_a(0)
            weight_sign(0, 0)          # blocks 0,1
            weight_transpose(0, 0)     # taps for pair 0
            xsign(0, 0)
            xsign(0, 1)
            weight_sign(0, 1)          # blocks 2,3
            g0 = [(0, 0), (0, 1)]
            g0_accs = [
                psum_mm.tile([128, NFREE], FP32, tag="acc", name="acc")
                for _ in g0
            ]
            conv_rounds(0, g0, g0_accs, 0)
            xsign(0, 2)
            xsign(0, 3)
            xsign(1, 0)
            xsign(1, 1)
            weight_transpose(0, 1)     # taps for pair 1
            weight_prep_c(0)
            conv_rounds(0, g0, g0_accs, 1)
            for j, (n, h2) in enumerate(g0):
                epilogue(0, n, h2, g0_accs[j])
            xsign(1, 2)
            xsign(1, 3)
            conv_group(0, [(1, 0), (1, 1)])
            for n in (2, 3):
                for c in range(NCH):
                    xsign(n, c)
            weight_prep_a(1)
            weight_sign(1, 0)
            weight_sign(1, 1)
            weight_transpose(1, 0)
            weight_transpose(1, 1)
            conv_group(0, [(2, 0), (2, 1), (3, 0), (3, 1)])
            weight_prep_c(1)

            # ---- main loop: conv(m) interleaved with prep(m+1) -----------
            # transposes for m+1 are emitted BEFORE the last conv group of
            # m so the PE/DVE work hides under conv and the m->m+1
            # transition has no pipeline bubble.
            for m in range(1, NCH):
                groups = [
                    [(0, 0), (0, 1), (1, 0), (1, 1)],
                    [(2, 0), (2, 1), (3, 0), (3, 1)],
                ]
                for gi, group in enumerate(groups):
                    last = gi == len(groups) - 1
                    if last and m + 1 < NCH:
                        weight_transpose(m + 1, 0)
                        weight_transpose(m + 1, 1)
                    conv_group(m, group)
                    if gi == 0 and m + 1 < NCH:
                        weight_prep_a(m + 1)
                        weight_sign(m + 1, 0)
                        weight_sign(m + 1, 1)
                    if last and m + 1 < NCH:
                        weight_prep_c(m + 1)

    nc.finalize()
    return nc


_NC_CACHE = None


def _get_program():
    global _NC_CACHE
    if _NC_CACHE is None:
        _NC_CACHE = build_program()
    return _NC_CACHE


def kernel(**inputs):
    from concourse.bass_utils import run_bass_kernel_spmd

    x = np.ascontiguousarray(np.asarray(inputs["x"], dtype=np.float32))
    shared = {
        name: np.ascontiguousarray(np.asarray(inputs[name], dtype=np.float32))
        for name in (
            "conv_weight", "gain", "move0_bias", "move1_bias", "prelu_a",
            "move2_bias",
        )
    }
    nc = _get_program()
    in_maps = [
        {"x": x[i * N_PER : (i + 1) * N_PER], **shared} for i in range(N_CORES)
    ]
    res = run_bass_kernel_spmd(nc, in_maps, core_ids=list(range(N_CORES)))
    return np.concatenate([r["out"] for r in res.results], axis=0)


# revision 20
# speedup vs baseline: 1.2202x; 1.2202x over previous
"""Trainium2 Bass kernel for nn_BasicBlock (binary activation + binarized
weight-standardized 3x3 conv + residual + PReLU).

Contract: kernel(**inputs) takes FULL unsharded numpy inputs (keys as in
setup_inputs) and returns the FULL [32, 512, 28, 28] float32 output.
Internally shards the batch dim across 8 NeuronCores (4 images each); the
small conv weight + per-channel vectors are replicated.

Key math facts exploited:
- forward activations are sign(x*beta+b0) in {-1,0,1} and forward weights
  are sf[o]*gain[o]*sign(w_std) with sign in {-1,0,1}; we encode acts as
  +-0.5 (is_gt trick on DVE/gpsimd, off the ACT engine) and weights as
  +-1, folding the 2x into the per-channel epilogue scalar alphabar.
- fp8e4 DoubleRow packs two contraction rows per PE cell (2 cin chunks per
  matmul), halving the matmul count; products are +-0.5 so fp32 PSUM
  accumulation is exact.
- startup: x(image 0)+w(chunk 0) DMAs are prioritized, junk transposes
  warm the PE HAM clock gate, and the weight-prep pipeline
  (stats -> sign -> transpose -> cast) is pipelined at (tap, pair)
  granularity so conv starts as soon as the first weight tap is ready.
"""

import numpy as np

import concourse.bass as bass
import concourse.mybir as mybir
import concourse.tile as tile
from concourse import bacc
from concourse.masks import make_identity

# problem constants (hardcoded per harness contract)
N_CORES = 8
N_PER = 4          # images per core (32 / 8)
C = 512            # Cin == Cout
H = W = 28
HP = WP = 30       # zero-padded spatial
TAPS = 9
KFAN = C * TAPS    # 4608 = fan-in per output channel
ALPHA = 0.2
BETA = 1.0
EPS = 1e-5
WS_SCALE = 1.0 / float(np.sqrt(KFAN))  # fan_in**-0.5
NCH = C // 128     # 4 channel chunks of 128
NPAIR = NCH // 2   # 2 DoubleRow pairs of chunks
ROWS_PER_TILE = 14 # output rows per matmul tile
NSPAT = H // ROWS_PER_TILE  # 2 spatial tiles per image
NFREE = ROWS_PER_TILE * WP  # 420: contiguous run incl. 2 pad cols per row
ACT_IMG = 912  # padded 30x30 image (900) + 12 slack: %16==0 for DoubleRow,
               # and covers the last tile's 420-run overhang (482+420=902)
NROUND = NPAIR * TAPS  # 18 accumulation rounds per output tile

FP32 = mybir.dt.float32
BF16 = mybir.dt.bfloat16
FP8 = mybir.dt.float8e4


def build_program():
    nc = bacc.Bacc(
        "TRN2",
        target_bir_lowering=False,
        debug=False,
        num_devices=1,
        num_swdge_queues=4,
    )
    x_h = nc.declare_dram_parameter("x", [N_PER, C, H, W], FP32, isOutput=False)
    w_h = nc.declare_dram_parameter("conv_weight", [C, C, 3, 3], FP32, isOutput=False)
    gain_h = nc.declare_dram_parameter("gain", [C], FP32, isOutput=False)
    b0_h = nc.declare_dram_parameter("move0_bias", [C], FP32, isOutput=False)
    b1_h = nc.declare_dram_parameter("move1_bias", [C], FP32, isOutput=False)
    pa_h = nc.declare_dram_parameter("prelu_a", [C], FP32, isOutput=False)
    b2_h = nc.declare_dram_parameter("move2_bias", [C], FP32, isOutput=False)
    out_h = nc.declare_dram_parameter("out", [N_PER, C, H, W], FP32, isOutput=True)

    x_ap = x_h[:, :, :, :]
    w_ap = w_h[:, :, :, :]
    out_ap = out_h[:, :, :, :]

    with tile.TileContext(nc) as tc:
        with (
            tc.tile_pool(name="persist", bufs=1) as persist,
            tc.tile_pool(name="scratch", bufs=2) as scratch,
            tc.tile_pool(name="stats", bufs=4) as stats,
            tc.tile_pool(name="epi", bufs=4) as epi,
            tc.tile_pool(name="psum_mm", bufs=6, space="PSUM") as psum_mm,
            tc.tile_pool(name="psum_tr", bufs=2, space="PSUM") as psum_tr,
        ):
            # ---- identity for PE transposes + HAM warm-up ---------------
            ident = persist.tile([128, 128], BF16, tag="ident")
            make_identity(nc, ident)

            # ---- activation image tiles + memsets (border zeros) --------
            # act_img[q][n] : [128, 2, ACT_IMG] fp8 -- two cin chunks per
            # DoubleRow pair; zeros at spatial border.
            # ---- small per-channel vectors first: cheap swdge DMAs -------
            def load_vec(h, nm):
                t = persist.tile([128, NCH], FP32, tag=f"v_{nm}", name=f"v_{nm}")
                nc.gpsimd.dma_start(
                    out=t, in_=h[:].rearrange("(c p) -> p c", p=128)
                )
                return t

            gain_v = load_vec(gain_h, "gain")
            b0_v = load_vec(b0_h, "b0")
            b1_v = load_vec(b1_h, "b1")
            pa_v = load_vec(pa_h, "pa")
            b2_v = load_vec(b2_h, "b2")
            gain_c = [gain_v[:, c : c + 1] for c in range(NCH)]
            b1_c = [b1_v[:, c : c + 1] for c in range(NCH)]
            pa_c = [pa_v[:, c : c + 1] for c in range(NCH)]
            b2_c = [b2_v[:, c : c + 1] for c in range(NCH)]

            # thr[c] = -b0[c]/beta : act sign threshold (x > thr)
            thr_v = persist.tile([128, NCH], FP32, tag="thr")
            nc.vector.tensor_scalar_mul(out=thr_v, in0=b0_v, scalar1=-1.0 / BETA)
            thr_c = [thr_v[:, c : c + 1] for c in range(NCH)]

            # derived per-channel epilogue constants:
            #   one_minus_a = 1 - prelu_a ; ab1b2 = prelu_a*move1_bias + move2_bias
            oma_v = persist.tile([128, NCH], FP32, tag="oma")
            nc.vector.tensor_scalar(
                out=oma_v, in0=pa_v, scalar1=-1.0, scalar2=1.0,
                op0=mybir.AluOpType.mult, op1=mybir.AluOpType.add,
            )
            one_minus_a = [oma_v[:, c : c + 1] for c in range(NCH)]
            ab_v = persist.tile([128, NCH], FP32, tag="ab1b2")
            for c in range(NCH):
                nc.vector.scalar_tensor_tensor(
                    out=ab_v[:, c : c + 1], in0=b1_c[c], scalar=pa_c[c],
                    in1=b2_c[c],
                    op0=mybir.AluOpType.mult, op1=mybir.AluOpType.add,
                )
            ab1b2 = [ab_v[:, c : c + 1] for c in range(NCH)]

            # ---- activation image tiles + memsets (border zeros) ---------
            # act_img[q][n] : [128, 2, ACT_IMG] fp8 -- two cin chunks per
            # DoubleRow pair; zeros at spatial border. Image 0/1 tiles
            # memset first (they gate the first xsigns).
            act_img = [[None] * N_PER for _ in range(NPAIR)]
            for n in range(N_PER):
                for q in range(NPAIR):
                    act_img[q][n] = persist.tile(
                        [128, 2, ACT_IMG], FP8, tag=f"act{q}_{n}", name=f"act{q}_{n}"
                    )
            for n in range(N_PER):
                for q in range(NPAIR):
                    nc.gpsimd.memset(act_img[q][n], 0.0)

            # ---- HAM warm-up: junk transposes keep PE busy pre-conv ------
            # (shares the psum_tr rotation via the same tag)
            junk_ps = psum_tr.tile([128, 2, 128], BF16, tag="ptr", name="junk")
            for _ in range(100):
                nc.tensor.transpose(junk_ps[:, 0, :], ident, ident)

            # ---- weight DMAs: chunk 0 split into 9 sg-aligned pieces
            # across the two HWDGE engines; chunks 1-3 as single DMAs -----
            w_flat = w_ap.rearrange("o i a b -> o (i a b)")
            w_tiles = [None] * NCH

            xs_tiles = [
                persist.tile([128, N_PER, H, W], FP32, tag=f"xs{c}", name=f"xs{c}")
                for c in range(NCH)
            ]
            xr = x_ap.rearrange("n c h w -> c n h w")

            def x_dma(eng, c, n0, n1):
                eng.dma_start(
                    out=xs_tiles[c][:, n0:n1],
                    in_=xr[c * 128 : (c + 1) * 128, n0:n1],
                )

            def w_dma_chunk(m):
                w_tiles[m] = scratch.tile(
                    [128, KFAN], FP32, tag="wtile", name=f"wt{m}"
                )
                nc.sync.dma_start(
                    out=w_tiles[m],
                    in_=w_flat[m * 128 : (m + 1) * 128, :],
                )

            # hw-queue priority order (sync engine = issue order):
            # x(img0 pair0), w0 pieces, x(img1 pair0), x(img0/1 pair1),
            # w1, x(img2/3), w2, w3.  w0's pieces 5-8 go via scalar so the
            # two HWDGE engines issue concurrently.
            w_tiles[0] = scratch.tile([128, KFAN], FP32, tag="wtile", name="wt0")
            x_dma(nc.sync, 0, 0, 1)
            x_dma(nc.sync, 1, 0, 1)
            for j in range(TAPS):
                eng = nc.sync if j < 5 else nc.scalar
                eng.dma_start(
                    out=w_tiles[0][:, j * 512 : (j + 1) * 512],
                    in_=w_flat[0:128, j * 512 : (j + 1) * 512],
                )
            x_dma(nc.sync, 0, 1, 2)
            x_dma(nc.sync, 1, 1, 2)
            x_dma(nc.sync, 2, 0, 2)
            x_dma(nc.sync, 3, 0, 2)
            w_dma_chunk(1)
            x_dma(nc.sync, 0, 2, 4)
            x_dma(nc.sync, 1, 2, 4)
            x_dma(nc.sync, 2, 2, 4)
            x_dma(nc.sync, 3, 2, 4)
            w_dma_chunk(2)
            w_dma_chunk(3)

            # ---- act sign: sign(beta*x + b0) -> fp8 on ACT ---------------
            b0_c = [b0_v[:, c : c + 1] for c in range(NCH)]

            def xsign(n, c):
                dst = act_img[c // 2][n][:, c % 2, : HP * WP].rearrange(
                    "p (h w) -> p h w", w=WP
                )[:, 1 : 1 + H, 1 : 1 + W]
                nc.scalar.activation(
                    out=dst,
                    in_=xs_tiles[c][:, n],
                    func=mybir.ActivationFunctionType.Sign,
                    bias=b0_c[c],
                    scale=BETA,
                )

            # lhsT : [128(cin), tap, pair, half, cout] fp8 DoubleRow weights
            lhsT = persist.tile(
                [128, TAPS, NPAIR, 2, C], FP8, tag="lhsT", name="lhsT"
            )
            alphabar = {}  # per cout chunk [128,1]: 2*alpha*sf*gain
            wsigns = {}
            mvs = {}

            def weight_prep_a(m):
                """stats -> negmean: the critical path to the signs"""
                wt = w_tiles[m]
                st = stats.tile([128, TAPS, 6], FP32, tag="bnst", name="bnst")
                wt3 = wt.rearrange("p (a b) -> p a b", b=512)
                for sg in range(TAPS):
                    nc.vector.bn_stats(out=st[:, sg, :], in_=wt3[:, sg, :])
                mv = stats.tile([128, 2], FP32, tag="bnagg", name="bnagg")
                nc.vector.bn_aggr(out=mv, in_=st)

                negmean = stats.tile([128, 1], FP32, tag="negmean", name="negmean")
                nc.vector.tensor_scalar_mul(out=negmean, in0=mv[:, 0:1], scalar1=-1.0)
                mvs[m] = (mv, negmean)

            def weight_sign(m, q):
                """sign(w - mean) -> bf16 (+-1) for pair q's two cin blocks"""
                wt = w_tiles[m]
                _, negmean = mvs[m]
                if q == 0:
                    ws = scratch.tile([128, KFAN], BF16, tag="wsign", name="wsign")
                    wsigns[m] = ws
                ws = wsigns[m]
                for h2 in range(2):
                    b = 2 * q + h2
                    nc.scalar.activation(
                        out=ws[:, b * 1152 : (b + 1) * 1152],
                        in_=wt[:, b * 1152 : (b + 1) * 1152],
                        func=mybir.ActivationFunctionType.Sign,
                        bias=negmean,
                    )

            def weight_transpose(m, q):
                """per tap: 2 block transposes -> 1 fp8 cast (pair q)"""
                ws3 = wsigns[m].rearrange("p (i t) -> p i t", t=TAPS)
                for t in range(TAPS):
                    ps = psum_tr.tile(
                        [128, 2, 128], BF16, tag="ptr", name="ptr"
                    )
                    for h2 in range(2):
                        b = 2 * q + h2
                        nc.tensor.transpose(
                            ps[:, h2, :],
                            ws3[:, b * 128 : (b + 1) * 128, t],
                            ident,
                        )
                    nc.vector.tensor_copy(
                        out=lhsT[:, t, q, :, m * 128 : (m + 1) * 128],
                        in_=ps,
                    )

            def weight_prep_c(m):
                """1/(std+eps), sum|w-mean| -> alphabar; off critical path"""
                wt = w_tiles[m]
                mv, negmean = mvs[m]
                stdeps = stats.tile([128, 1], FP32, tag="stdeps", name="stdeps")
                nc.scalar.activation(
                    out=stdeps, in_=mv[:, 1:2], func=mybir.ActivationFunctionType.Sqrt
                )
                nc.vector.tensor_scalar_add(out=stdeps, in0=stdeps, scalar1=EPS)
                inv = stats.tile([128, 1], FP32, tag="inv", name="inv")
                nc.vector.reciprocal(out=inv, in_=stdeps)

                sumabs = stats.tile([128, NCH], FP32, tag="sumabs", name="sumabs")
                for b in range(NCH):
                    nc.scalar.activation(
                        out=wt[:, b * 1152 : (b + 1) * 1152],
                        in_=wt[:, b * 1152 : (b + 1) * 1152],
                        func=mybir.ActivationFunctionType.Abs,
                        bias=negmean,
                        accum_out=sumabs[:, b : b + 1],
                    )
                sumabs1 = stats.tile([128, 1], FP32, tag="sumabs1", name="sumabs1")
                nc.vector.tensor_reduce(
                    out=sumabs1, in_=sumabs, axis=mybir.AxisListType.X,
                    op=mybir.AluOpType.add,
                )

                ab = persist.tile(
                    [128, 1], FP32, tag=f"alphabar{m}", name=f"alphabar{m}"
                )
                nc.vector.tensor_tensor(
                    out=ab, in0=sumabs1, in1=inv, op=mybir.AluOpType.mult
                )
                nc.vector.tensor_tensor(
                    out=ab, in0=ab, in1=gain_c[m], op=mybir.AluOpType.mult
                )
                nc.vector.tensor_scalar_mul(
                    out=ab, in0=ab, scalar1=ALPHA * WS_SCALE / KFAN
                )
                alphabar[m] = ab

            def epilogue(m, n, h2, acc):
                y0 = h2 * ROWS_PER_TILE
                accv = acc.rearrange("p (h w) -> p h w", w=WP)[:, :, 0:W]
                res = xs_tiles[m][:, n, y0 : y0 + ROWS_PER_TILE, :]
                # z = acc*alphabar + residual   (prelu input minus b1)
                z = epi.tile(
                    [128, ROWS_PER_TILE, W], FP32, tag="z", name="z"
                )
                nc.vector.scalar_tensor_tensor(
                    out=z, in0=accv, scalar=alphabar[m], in1=res,
                    op0=mybir.AluOpType.mult, op1=mybir.AluOpType.add,
                )
                # r = relu(z + b1) on ACT
                r = epi.tile(
                    [128, ROWS_PER_TILE, W], FP32, tag="r", name="r"
                )
                nc.scalar.activation(
                    out=r, in_=z,
                    func=mybir.ActivationFunctionType.Relu,
                    bias=b1_c[m],
                )
                # zz = a*z + (a*b1 + b2) ; out = (1-a)*r + zz
                zz = epi.tile(
                    [128, ROWS_PER_TILE, W], FP32, tag="zz", name="zz"
                )
                nc.scalar.activation(
                    out=zz, in_=z,
                    func=mybir.ActivationFunctionType.Identity,
                    scale=pa_c[m], bias=ab1b2[m],
                )
                nc.vector.scalar_tensor_tensor(
                    out=zz, in0=r, scalar=one_minus_a[m], in1=zz,
                    op0=mybir.AluOpType.mult, op1=mybir.AluOpType.add,
                )
                nc.sync.dma_start(
                    out=out_ap[
                        n, m * 128 : (m + 1) * 128,
                        y0 : y0 + ROWS_PER_TILE, :,
                    ],
                    in_=zz,
                )

            def conv_rounds(m, group, accs, q):
                """emit the 9 accumulation rounds of pair q for a group"""
                for t in range(TAPS):
                    dy, dx = t // 3, t % 3
                    wslice = lhsT[:, t, q, :, m * 128 : (m + 1) * 128]
                    for j, (n, h2) in enumerate(group):
                        base = (h2 * ROWS_PER_TILE + dy) * WP + dx
                        rhs = act_img[q][n][:, :, base : base + NFREE]
                        nc.tensor.matmul(
                            accs[j],
                            wslice,
                            rhs,
                            start=(q == 0 and t == 0),
                            stop=(q == NPAIR - 1 and t == TAPS - 1),
                            perf_mode=mybir.MatmulPerfMode.DoubleRow,
                        )

            def conv_group(m, group):
                """one PSUM-resident group: 18 rounds, weight reused
                across the group's tiles within each round"""
                accs = [
                    psum_mm.tile([128, NFREE], FP32, tag="acc", name="acc")
                    for _ in group
                ]
                for q in range(NPAIR):
                    conv_rounds(m, group, accs, q)
                for j, (n, h2) in enumerate(group):
                    epilogue(m, n, h2, accs[j])

            # ---- chunk 0 startup pipeline: conv on image 0 starts as
            # soon as pair-0 weights + image-0 pair-0 acts are ready; the
            # pair-1 weight prep and remaining xsigns hide under the q0
            # conv rounds. ------------------------------------------------
            weight_prep_a(0)
            weight_sign(0, 0)          # blocks 0,1
            weight_transpose(0, 0)     # taps for pair 0
            xsign(0, 0)
            xsign(0, 1)
            weight_sign(0, 1)          # blocks 2,3
            g0 = [(0, 0), (0, 1)]
            g0_accs = [
                psum_mm.tile([128, NFREE], FP32, tag="acc", name="acc")
                for _ in g0
            ]
            conv_rounds(0, g0, g0_accs, 0)
            xsign(0, 2)
            xsign(0, 3)
            xsign(1, 0)
            xsign(1, 1)
            weight_transpose(0, 1)     # taps for pair 1
            weight_prep_c(0)
            conv_rounds(0, g0, g0_accs, 1)
            for j, (n, h2) in enumerate(g0):
                epilogue(0, n, h2, g0_accs[j])
            xsign(1, 2)
            xsign(1, 3)
            conv_group(0, [(1, 0), (1, 1)])
            for c in range(NCH):
                xsign(2, c)
            conv_group(0, [(2, 0), (2, 1)])
            for c in range(NCH):
                xsign(3, c)
            weight_prep_a(1)
            weight_sign(1, 0)
            weight_sign(1, 1)
            weight_transpose(1, 0)
            weight_transpose(1, 1)
            conv_group(0, [(3, 0), (3, 1)])
            weight_prep_c(1)

            # ---- main loop: conv(m) interleaved with prep(m+1) -----------
            # groups of 2 PSUM tiles: with bufs=6, a group's banks were
            # freed 2 groups (about 7 us) before it starts, so its tiles
            # never skew apart and each weight load serves the whole
            # round back-to-back. transposes for m+1 are emitted BEFORE
            # the last groups of m so they hide under conv.
            for m in range(1, NCH):
                conv_group(m, [(0, 0), (0, 1)])
                if m + 1 < NCH:
                    weight_prep_a(m + 1)
                    weight_sign(m + 1, 0)
                    weight_sign(m + 1, 1)
                conv_group(m, [(1, 0), (1, 1)])
                if m + 1 < NCH:
                    weight_transpose(m + 1, 0)
                conv_group(m, [(2, 0), (2, 1)])
                if m + 1 < NCH:
                    weight_transpose(m + 1, 1)
                conv_group(m, [(3, 0), (3, 1)])
                if m + 1 < NCH:
                    weight_prep_c(m + 1)

    nc.finalize()
    return nc


_NC_CACHE = None


def _get_program():
    global _NC_CACHE
    if _NC_CACHE is None:
        _NC_CACHE = build_program()
    return _NC_CACHE


def kernel(**inputs):
    from concourse.bass_utils import run_bass_kernel_spmd

    x = np.ascontiguousarray(np.asarray(inputs["x"], dtype=np.float32))
    shared = {
        name: np.ascontiguousarray(np.asarray(inputs[name], dtype=np.float32))
        for name in (
            "conv_weight", "gain", "move0_bias", "move1_bias", "prelu_a",
            "move2_bias",
        )
    }
    nc = _get_program()
    in_maps = [
        {"x": x[i * N_PER : (i + 1) * N_PER], **shared} for i in range(N_CORES)
    ]
    res = run_bass_kernel_spmd(nc, in_maps, core_ids=list(range(N_CORES)))
    return np.concatenate([r["out"] for r in res.results], axis=0)
